# revision 9
# baseline (speedup 1.0000x reference)
"""BoundaryLoss Trainium2 kernel V3 (8-core data-parallel).

Math/screen as V1: loss*N = sum(lse) - sum(x_t) - (1-c1)*corr over the <=9
non-boundary pixels (host-corrected), else exact numpy fallback.

V3 engine plan (per core, 2 images), all x as fp8-e4m3:
 - exp: ACT, fp8 in -> fp8 out, class-pair-contiguous tiles.
 - se: PE DoubleRow fp8 identity matmuls (0.5 cyc/row) into PSUM.
 - lse: ACT Ln over 2-strip PSUM spans, accum columns.
 - x_t: 16x scalar_tensor_tensor (t==c)*x_c with accum on DVE (the only
   engine whose ISA supports it).
 - boundary: eh/ev on DVE; h2 (horiz 2-sum of eh) on gpsimd; vertical sums
   on PE -- T3/U/D bands over h2 plus T2/U bands over ev at 3 horizontal
   rhs shifts (replaces the h3/tmp DVE ops); b = Sign(sum - .5) on ACT,
   exported as fp8 (+1 boundary / -1 not).
"""
import math
from contextlib import nullcontext as _nullcontext
import numpy as np
import ml_dtypes
import concourse.bass as bass
import concourse.tile as tile
from concourse import mybir
from concourse.bass_utils import run_bass_kernel_spmd

BF16 = mybir.dt.bfloat16
F32 = mybir.dt.float32
FP8 = mybir.dt.float8e4
AF = mybir.ActivationFunctionType
OP = mybir.AluOpType
DR = mybir.MatmulPerfMode.DoubleRow

B, C, H, W = 16, 8, 512, 512
N_CORES = 8
PER = B // N_CORES
S = H // 128
SW = S * W
THETA = 5.0
MAX_ITERS = 15
C1 = math.exp(-1.0 / THETA)
NPIX = B * H * W
NPAIR = C // 2

# accumulator columns per image: 2 lse (2-strip spans) + 8 xt (per class)
NLSE = 1
COLS_PER_IMG = NLSE + C
NCOLS = PER * COLS_PER_IMG

H2_ON_POOL = True   # gpsimd tensor_tensor for the eh 2-sum; False -> DVE
EV_ON_PE = True     # ev 3-tap via shifted PE bands; False -> DVE h3/tmp + T2/U bands


def _split_sync_waits(nc, max_waits=1):
    """Walrus CoreV3 codegen rejects >1 sync wait per instruction; hoist
    extras onto NoOps inserted just before."""
    k = 0
    for f in nc.m.functions:
        for bb in f.blocks:
            new = []
            for ins in bb.instructions:
                w = list(ins.sync_info.on_wait) if ins.sync_info else []
                if len(w) > max_waits:
                    extra, keep = w[:-max_waits], w[-max_waits:]
                    for s0 in range(0, len(extra), max_waits):
                        nop = mybir.InstNoOp(
                            name=f"I-wsplit-{k}", ins=[], outs=[],
                            sync_info=mybir.SyncInfo(
                                on_wait=extra[s0:s0 + max_waits], on_update=[]),
                            engine=ins.engine)
                        k += 1
                        new.append(nop)
                    ins.sync_info.on_wait = keep
                new.append(ins)
            bb.instructions = new


def _band_consts():
    """bf16 [128, 4*128]: T3 (tridiag), T2 (k in {p-1,p}), U (k=127 -> p=0),
    D (k=0 -> p=127). lhsT layout: [k, p]."""
    k = np.arange(128)[:, None]
    p = np.arange(128)[None, :]
    T3 = (np.abs(k - p) <= 1).astype(np.float32)
    T2 = ((k == p) | (k == p - 1)).astype(np.float32)
    U = ((k == 127) & (p == 0)).astype(np.float32)
    D = ((k == 0) & (p == 127)).astype(np.float32)
    return np.concatenate([T3, T2, U, D], axis=1).astype(ml_dtypes.bfloat16)


def _pair_id_fp8():
    """fp8 [128, 2*128]: identity in both DoubleRow slots (slot-major)."""
    k = np.arange(128)[:, None]
    p = np.arange(128)[None, :]
    I = (k == p).astype(np.float32)
    return np.concatenate([I, I], axis=1).astype(ml_dtypes.float8_e4m3fn)


_NC_CACHE = {}


def _blk(ap):
    return ap.rearrange("p (s w) -> p s w", s=S)


def _stk(dram_img):
    return dram_img.rearrange("(s p) w -> p s w", p=128)


def _build_nc(repeat=1, split=True, loop_rep=0):
    key = (repeat, split, loop_rep)
    if key in _NC_CACHE:
        return _NC_CACHE[key]
    nc = bass.Bass()
    xq = nc.dram_tensor("xq", [PER, C, H, W], FP8, kind="ExternalInput")
    tg = nc.dram_tensor("tg", [PER, H, W], BF16, kind="ExternalInput")
    cst = nc.dram_tensor("cst", [128, 4 * 128], BF16, kind="ExternalInput")
    cstq = nc.dram_tensor("cstq", [128, 2 * 128], FP8, kind="ExternalInput")
    out = nc.dram_tensor("out", [128, NCOLS], F32, kind="ExternalOutput")
    bm = nc.dram_tensor("bm", [PER, H, W], FP8, kind="ExternalOutput")

    with tile.TileContext(nc) as tc:
        with (
            tc.tile_pool(name="pc", bufs=1) as pc,
            tc.tile_pool(name="pp", bufs=1) as pp,
            tc.tile_pool(name="pt", bufs=1) as pt,
            tc.tile_pool(name="px", bufs=2) as px,
            tc.tile_pool(name="pa", bufs=1) as pa,
            tc.tile_pool(name="ps", bufs=2, space="PSUM") as ps,
            tc.tile_pool(name="ps1", bufs=1, space="PSUM") as ps1,
        ):
            cons = pc.tile([128, 4 * 128], BF16, tag="cons")
            nc.sync.dma_start(cons[:], cst[:])
            T3 = cons[:, 0:128]
            T2 = cons[:, 128:256]
            Uc = cons[:, 256:384]
            Dc = cons[:, 384:512]
            consq = pc.tile([128, 2 * 128], FP8, tag="consq")
            nc.sync.dma_start(consq[:], cstq[:])
            IpairQ = consq[:].rearrange("p (two m) -> p two m", two=2)

            cols = pa.tile([128, NCOLS], F32, tag="cols")
            bneg = pc.tile([128, 1], F32, tag="bneg")
            nc.vector.memset(bneg[:], -0.5)

            loop_cm = tc.For_i(0, loop_rep, 1) if loop_rep > 0 else _nullcontext()
            with loop_cm:
                for rep in range(repeat):
                    imgs = list(range(PER))
                    t_t, td_t, xq_t, eq_t = {}, {}, {}, {}
                    eh_t, ev_t, h2_t = {}, {}, {}
                    # ---- DMAs ----
                    for img in imgs:
                        t = pp.tile([128, SW], BF16, tag=f"t{img}", name=f"t{img}")
                        t_t[img] = t
                        nc.sync.dma_start(_blk(t[:]), _stk(tg[img]))
                        xqt = px.tile([128, C * SW], FP8, tag=f"xq{img}",
                                      name=f"xq{img}")
                        xq_t[img] = xqt
                        for c in range(C):
                            nc.sync.dma_start(
                                _blk(xqt[:, c * SW:(c + 1) * SW]), _stk(xq[img, c]))
                        td = pt.tile([128, SW], BF16, tag=f"td{img}", name=f"td{img}")
                        td_t[img] = td
                        nc.sync.dma_start(
                            td.rearrange("p (s w) -> p s w", s=S)[:, 0:S - 1, :],
                            tg[img, 1:H - 127, :].rearrange("(s p) w -> p s w", p=128))
                        nc.sync.dma_start(td[0:127, (S - 1) * W:S * W],
                                          tg[img, (S - 1) * 128 + 1:H, :])
                        nc.sync.dma_start(td[127:128, (S - 1) * W:S * W],
                                          tg[img, H - 1:H, :])

                    # ---- edge maps ----
                    for img in imgs:
                        t, td = t_t[img], td_t[img]
                        tb = _blk(t[:])
                        eh = pt.tile([128, SW], BF16, tag=f"eh{img}", name=f"eh{img}")
                        eh_t[img] = eh
                        ehb = _blk(eh[:])
                        nc.gpsimd.memset(ehb[:, :, W - 1:W], 0.0)
                        nc.vector.tensor_tensor(out=ehb[:, :, 0:W - 1],
                                                in0=tb[:, :, 0:W - 1],
                                                in1=tb[:, :, 1:W], op=OP.not_equal)
                        ev = pt.tile([128, SW], BF16, tag=f"ev{img}", name=f"ev{img}")
                        ev_t[img] = ev
                        nc.vector.tensor_tensor(out=ev[:], in0=t[:], in1=td[:],
                                                op=OP.not_equal)
                        if not EV_ON_PE:
                            h3 = pp.tile([128, SW], BF16, tag=f"h3{img}",
                                         name=f"h3{img}")
                            h3b = _blk(h3[:])
                            evb = _blk(ev[:])
                            tmp = pt.tile([128, SW], BF16, tag=f"tm{img}",
                                          name=f"tm{img}")
                            tmpb = _blk(tmp[:])
                            nc.vector.tensor_tensor(out=tmpb[:, :, 0:W - 1],
                                                    in0=evb[:, :, 0:W - 1],
                                                    in1=evb[:, :, 1:W], op=OP.add)
                            nc.vector.tensor_tensor(out=h3b[:, :, 1:W - 1],
                                                    in0=tmpb[:, :, 0:W - 2],
                                                    in1=evb[:, :, 2:W], op=OP.add)
                            nc.gpsimd.tensor_copy(h3b[:, :, 0:1], tmpb[:, :, 0:1])
                            nc.gpsimd.tensor_copy(h3b[:, :, W - 1:W],
                                                  tmpb[:, :, W - 2:W - 1])
                            ev_t[img] = h3
                        # h2[w] = eh[w-1] + eh[w]; h2[0] = eh[0]
                        h2 = pp.tile([128, SW], BF16, tag=f"h2{img}", name=f"h2{img}")
                        h2_t[img] = h2
                        h2b = _blk(h2[:])
                        nc.gpsimd.tensor_copy(h2b[:, :, 0:1], ehb[:, :, 0:1])
                        eng = nc.gpsimd if H2_ON_POOL else nc.vector
                        eng.tensor_tensor(out=h2b[:, :, 1:W], in0=ehb[:, :, 0:W - 1],
                                          in1=ehb[:, :, 1:W], op=OP.add)

                    # ---- exp (ACT pairs, fp8 in/out) ----
                    for img in imgs:
                        xqt = xq_t[img]
                        eq = px.tile([128, C * SW], FP8, tag=f"eq{img}",
                                     name=f"eq{img}")
                        eq_t[img] = eq
                        for j in range(2):
                            nc.scalar.activation(
                                eq[:, 4 * j * SW:(4 * j + 4) * SW],
                                xqt[:, 4 * j * SW:(4 * j + 4) * SW], AF.Exp)

                    # ---- boundary: PE bands + ACT Sign + bm out ----
                    for img in imgs:
                        h2, ev = h2_t[img], ev_t[img]
                        bt = pt.tile([128, SW], FP8, tag=f"bt{img}", name=f"bt{img}")
                        for s in range(S):
                            c0, c1_ = s * W, (s + 1) * W
                            sb = ps.tile([128, W], F32, tag="sb")
                            # eh-part: vertical 3 over h2 (horizontal 2-sum)
                            nc.tensor.matmul(sb[:], T3, h2[:, c0:c1_],
                                             start=True, stop=False)
                            if s > 0:
                                nc.tensor.matmul(sb[:], Uc, h2[:, c0 - W:c0],
                                                 start=False, stop=False)
                            if s < S - 1:
                                nc.tensor.matmul(sb[:], Dc, h2[:, c1_:c1_ + W],
                                                 start=False, stop=False)
                            # ev-part: vertical 2 (T2 + U halo) at 3 horiz
                            # shifts; the full-range T2 dx=0 goes last to
                            # close the accumulation group.
                            if EV_ON_PE:
                                for band, src_off in ((Uc, -W), (T2, 0)):
                                    if src_off == -W and s == 0:
                                        continue
                                    b0 = c0 + src_off
                                    b1 = c1_ + src_off
                                    nc.tensor.matmul(sb[:, 1:W], band, ev[:, b0:b1 - 1],
                                                     start=False, stop=False)
                                    nc.tensor.matmul(sb[:, 0:W - 1], band, ev[:, b0 + 1:b1],
                                                     start=False, stop=False)
                                    nc.tensor.matmul(sb[:], band, ev[:, b0:b1],
                                                     start=False, stop=(band is T2))
                            else:
                                if s > 0:
                                    nc.tensor.matmul(sb[:], Uc, ev[:, c0 - W:c0],
                                                     start=False, stop=False)
                                nc.tensor.matmul(sb[:], T2, ev[:, c0:c1_],
                                                 start=False, stop=True)
                            nc.scalar.activation(bt[:, c0:c1_], sb[:], AF.Sign,
                                                 bias=bneg[:])
                        nc.sync.dma_start(_stk(bm[img]), _blk(bt[:]))

                    # ---- se accumulation (PSUM) + Ln ----
                    for img in imgs:
                        base = img * COLS_PER_IMG
                        eq = eq_t[img]
                        lse_scr = pt.tile([128, SW], BF16, tag=f"ls{img}",
                                          name=f"ls{img}")
                        se = ps1.tile([128, SW], F32, tag="seA",
                                      name=f"se{img}")
                        for s in range(S):
                            sl = slice(s * W, (s + 1) * W)
                            dst = se[:, s * W:(s + 1) * W]
                            for j in range(NPAIR):
                                nc.tensor.matmul(
                                    dst, IpairQ,
                                    eq[:, 2 * j * SW:(2 * j + 2) * SW]
                                    .rearrange("p (two f) -> p two f", two=2)[:, :, sl],
                                    start=(j == 0), stop=(j == NPAIR - 1),
                                    perf_mode=DR)
                        nc.scalar.activation(
                            lse_scr[:], se[:], AF.Ln,
                            accum_out=cols[:, base:base + 1])

                    # ---- x_t stt on DVE (filler, lowest priority) ----
                    scrD = pt.tile([128, SW], BF16, tag="scrD")
                    for img in imgs:
                        base = img * COLS_PER_IMG
                        t, xqt = t_t[img], xq_t[img]
                        for c in range(C):
                            nc.vector.scalar_tensor_tensor(
                                out=scrD[:], in0=t[:], scalar=float(c),
                                in1=xqt[:, c * SW:(c + 1) * SW],
                                op0=OP.is_equal, op1=OP.mult,
                                accum_out=cols[:, base + NLSE + c:base + NLSE + c + 1])

            nc.sync.dma_start(out[:], cols[:])

    if loop_rep > 0:
        for f in nc.m.functions:
            for bb in f.blocks:
                bb.instructions = [
                    i for i in bb.instructions
                    if getattr(i, "op_name", None) != "EVENT_SEMAPHORE_RANGE_CLEAR"
                ]
    if split:
        _split_sync_waits(nc)
    _NC_CACHE[key] = nc
    return nc


def _host_reduce(results, x=None, t=None):
    nb_idx = []
    tot_lse = tot_xt = 0.0
    for core, r in enumerate(results):
        bmap = r["bm"].view(ml_dtypes.float8_e4m3fn).astype(np.float32) \
            if r["bm"].dtype != ml_dtypes.float8_e4m3fn else r["bm"].astype(np.float32)
        for (ii, rr, cc) in np.argwhere(bmap < 0):
            nb_idx.append((core * PER + int(ii), int(rr), int(cc)))
            if len(nb_idx) >= 9:
                return 0.0, False
        cols = r["out"].astype(np.float64)
        for img in range(PER):
            base = img * COLS_PER_IMG
            tot_lse += cols[:, base:base + NLSE].sum()
            tot_xt += cols[:, base + NLSE:base + NLSE + C].sum()
    s_ce = tot_lse - tot_xt
    corr = 0.0
    if nb_idx and x is not None:
        for (gi, rr, cc) in nb_idx:
            v = x[gi, :, rr, cc].astype(np.float64)
            lse = math.log(np.exp(v).sum())
            corr += lse - v[int(t[gi, rr, cc])]
    loss = (s_ce - (1.0 - C1) * corr) / NPIX
    return loss, True


def _pool3(a, op):
    pad = -np.inf if op is np.maximum else np.inf
    p = np.pad(a, ((0, 0), (1, 1), (1, 1)), constant_values=pad)
    r = a.copy()
    for dy in (-1, 0, 1):
        for dx in (-1, 0, 1):
            r = op(r, p[:, 1 + dy:H + 1 + dy, 1 + dx:W + 1 + dx])
    return r


def _fallback(x, t):
    tf = t.astype(np.float32)
    bnd = (_pool3(tf, np.maximum) != _pool3(tf, np.minimum)).astype(np.float32)
    dist = np.zeros_like(bnd)
    cur = bnd.copy()
    for i in range(MAX_ITERS):
        dil = _pool3(cur, np.maximum)
        dist += (dil > cur).astype(np.float32) * (i + 1)
        cur = dil
    wts = np.exp(-dist / THETA)
    xm = x.max(axis=1, keepdims=True)
    lse = np.log(np.exp(x - xm).sum(axis=1)) + xm[:, 0]
    xt = np.take_along_axis(x, t[:, None].astype(np.int64), axis=1)[:, 0]
    return np.float32(np.mean((wts * (lse - xt)).astype(np.float64)))


def _in_maps(x, t):
    x8 = x.astype(ml_dtypes.float8_e4m3fn)
    tb = t.astype(ml_dtypes.bfloat16)
    return [
        {"xq": x8[i * PER:(i + 1) * PER],
         "tg": tb[i * PER:(i + 1) * PER],
         "cst": _band_consts(), "cstq": _pair_id_fp8()}
        for i in range(N_CORES)
    ]


def kernel(inputs, targets):
    x = np.ascontiguousarray(np.asarray(inputs))
    t = np.asarray(targets)
    nc = _build_nc()
    res = run_bass_kernel_spmd(nc, _in_maps(x, t), list(range(N_CORES)))
    loss, ok = _host_reduce(res.results, x, t)
    if not ok:
        return _fallback(x, t)
    return np.float32(loss)
